# revision 2
# baseline (speedup 1.0000x reference)
"""Causal self-attention (GPT-2 style: B=8, T=1024, C=768, H=12) on 8 TRN2 NeuronCores.

Strategy: data-parallel over batch (1 batch element per core, no collectives).

Per-core pipeline (all matmuls fp32r: ~fp32 accuracy at bf16-class PE throughput):
  1. QKV projection, with q,k produced TRANSPOSED ([C, T] layout, C on partitions)
     and v produced NATURAL ([T, C] layout) with a ones-column appended per head.
  2. Attention per head via TRANSPOSED scores: S^T[k,q] = (k^T).T @ q^T, so the
     causal mask is applied post-exp with an upper-triangular multiply on the
     single diagonal block, and the softmax denominator comes out of the same
     matmul as the output: lhsT = [v_head | ones] gives y_un^T rows 0..63 and
     the denominator row at 64. No max-subtraction (scores are bounded ~|2|).
  3. Normalization: 1/denominator broadcast across partitions via a K=1 matmul
     against a ones row, multiplied into y_un^T while copying PSUM->SBUF.
  4. Output projection from the y^T layout, bias added on PSUM->SBUF copy.
"""
import numpy as np

B, T, C, H = 8, 1024, 768, 12
HD = C // H  # 64
NT = T // 128  # 8 sequence tiles
NKC = C // 128  # 6 contraction chunks

_CACHE = {}


def _build():
    import concourse.bass as bass
    import concourse.mybir as mybir
    import concourse.tile as tile
    from concourse import bacc

    F32 = mybir.dt.float32
    F32R = mybir.dt.float32r
    AF = mybir.ActivationFunctionType

    nc = bacc.Bacc()
    xT = nc.declare_dram_parameter("xT", [C, T], F32R, isOutput=False)
    w_attn = nc.declare_dram_parameter("w_attn", [C, 3 * C], F32R, isOutput=False)
    bqk = nc.declare_dram_parameter("bqk", [128, 2 * C // 128], F32, isOutput=False)
    bv = nc.declare_dram_parameter("bv", [C], F32, isOutput=False)
    w_proj = nc.declare_dram_parameter("w_proj", [C, C], F32R, isOutput=False)
    bproj = nc.declare_dram_parameter("bproj", [C], F32, isOutput=False)
    tri = nc.declare_dram_parameter("tri", [128, 128], F32, isOutput=False)
    out = nc.declare_dram_parameter("out", [T, C], F32, isOutput=True)

    def pieces(c0):
        """Split columns [c0, 1024) at the 512 PSUM-bank boundary."""
        if c0 < 512:
            return [(c0, 512), (512, 1024)]
        return [(c0, 1024)]

    with tile.TileContext(nc) as tc:
        from contextlib import ExitStack
        with ExitStack() as ctx:
            const = ctx.enter_context(tc.tile_pool(name="const", bufs=1))
            qkp = ctx.enter_context(tc.tile_pool(name="qkp", bufs=1))
            vp = ctx.enter_context(tc.tile_pool(name="vp", bufs=1))
            psum = ctx.enter_context(tc.tile_pool(name="psum", bufs=2, space="PSUM"))

            # ---- constants ----
            bqk_sb = const.tile([128, 2 * C // 128], F32, tag="bqk")
            nc.sync.dma_start(out=bqk_sb, in_=bqk[:, :])
            bv_bc = const.tile([128, H, HD], F32, tag="bv")
            nc.gpsimd.dma_start(
                out=bv_bc,
                in_=bass.AP(tensor=bv, offset=0, ap=[[0, 128], [HD, H], [1, HD]]),
            )
            bproj_bc = const.tile([128, C], F32, tag="bproj")
            nc.gpsimd.dma_start(
                out=bproj_bc, in_=bass.AP(tensor=bproj, offset=0, ap=[[0, 128], [1, C]])
            )
            tri_sb = const.tile([128, 128], F32, tag="tri")
            nc.sync.dma_start(out=tri_sb, in_=tri[:, :])
            ones_f = const.tile([1, 128], F32, tag="onesf")
            nc.vector.memset(ones_f, 1.0)
            ones_sb = const.tile([1, 128], F32R, tag="ones")
            nc.vector.tensor_copy(ones_sb, ones_f)
            ones12 = const.tile([128, H, 1], F32, tag="ones12")
            nc.vector.memset(ones12, 1.0)

            # ---- persistent activations ----
            qkT = [qkp.tile([128, T], F32R, tag=f"qkT{m}", name=f"qkT{m}") for m in range(2 * C // 128)]
            v_sb = [vp.tile([128, H, HD + 1], F32R, tag=f"v{t}", name=f"v{t}") for t in range(NT)]

            # ---- phase 1: QKV ----
            with tc.tile_pool(name="wx", bufs=1) as wx:
                w_sb = []
                xT_sb = []
                for k in range(NKC):
                    w = wx.tile([128, 3 * C], F32R, tag=f"w{k}")
                    nc.sync.dma_start(out=w, in_=w_attn[128 * k:128 * (k + 1), :])
                    w_sb.append(w)
                    x = wx.tile([128, T], F32R, tag=f"x{k}")
                    nc.sync.dma_start(out=x, in_=xT[128 * k:128 * (k + 1), :])
                    xT_sb.append(x)

                # 1a: q^T and k^T tiles: out[m-block of 2C, T]
                for m in range(2 * C // 128):
                    ps = psum.tile([128, T], F32, tag="s")
                    for k in range(NKC):
                        for a, b in ((0, 512), (512, 1024)):
                            nc.tensor.matmul(
                                ps[:, a:b],
                                w_sb[k][:, 128 * m:128 * (m + 1)],
                                xT_sb[k][:, a:b],
                                start=(k == 0), stop=(k == NKC - 1),
                            )
                    nc.scalar.activation(
                        qkT[m], ps[:, :], AF.Identity,
                        bias=bqk_sb[:, m:m + 1], scale=1.0,
                    )

                # 1b: v natural tiles: out[t-block of T, C_v]
                for t in range(NT):
                    ps = psum.tile([128, C], F32, tag="y")
                    for k in range(NKC):
                        for a, b in ((0, 512), (512, C)):
                            nc.tensor.matmul(
                                ps[:, a:b],
                                xT_sb[k][:, 128 * t:128 * (t + 1)],
                                w_sb[k][:, 2 * C + a:2 * C + b],
                                start=(k == 0), stop=(k == NKC - 1),
                            )
                    nc.vector.tensor_add(
                        v_sb[t][:, :, 0:HD],
                        ps[:, :].rearrange("p (h d) -> p h d", d=HD),
                        bv_bc,
                    )
                    nc.vector.tensor_copy(v_sb[t][:, :, HD:HD + 1], ones12)

            # ---- phase 2: attention per head ----
            with tc.tile_pool(name="pp", bufs=3) as pp, \
                 tc.tile_pool(name="rp", bufs=2) as rp, \
                 tc.tile_pool(name="bcp", bufs=2) as bcp, \
                 tc.tile_pool(name="yout", bufs=1) as yout:
                ypair = [yout.tile([128, T], F32R, tag=f"yp{p}", name=f"yp{p}") for p in range(H // 2)]
                for h in range(H):
                    mq, off = h // 2, 64 * (h % 2)
                    mk = C // 128 + h // 2
                    y_ps = psum.tile([128, T], F32, tag="y")
                    for j in range(NT):
                        c0 = 128 * j
                        s_ps = psum.tile([128, T], F32, tag="s")
                        for a, b in pieces(c0):
                            nc.tensor.matmul(
                                s_ps[:, a:b],
                                qkT[mk][off:off + 64, c0:c0 + 128],
                                qkT[mq][off:off + 64, a:b],
                                start=True, stop=True,
                            )
                        p_sb = pp.tile([128, T], F32R, tag="p")
                        nc.scalar.activation(
                            p_sb[:, c0:T], s_ps[:, c0:T], AF.Exp, scale=0.125
                        )
                        nc.vector.tensor_mul(
                            p_sb[:, c0:c0 + 128], p_sb[:, c0:c0 + 128], tri_sb
                        )
                        for a, b in pieces(c0):
                            nc.tensor.matmul(
                                y_ps[0:HD + 1, a:b],
                                v_sb[j][:, h:h + 1, :],
                                p_sb[:, a:b],
                                start=(j == 0), stop=(j == NT - 1),
                                skip_group_check=True,
                            )
                    # normalize: bcast 1/denominator over partitions via K=1 matmul
                    r_sb = rp.tile([1, T], F32R, tag="r")
                    with nc.allow_low_precision(reason="fp32r recip of softmax denom"):
                        nc.vector.reciprocal(r_sb[0:1, :], y_ps[HD:HD + 1, :])
                    bc_ps = psum.tile([128, T], F32, tag="s")
                    for a in (0, 512):
                        nc.tensor.matmul(
                            bc_ps[0:64, a:a + 512], ones_sb[0:1, 0:64],
                            r_sb[0:1, a:a + 512], start=True, stop=True,
                        )
                    bc_sb = bcp.tile([64, T], F32, tag="bc")
                    nc.vector.tensor_copy(bc_sb, bc_ps[0:64, :])
                    nc.vector.tensor_mul(
                        ypair[h // 2][off:off + 64, :], y_ps[0:64, :], bc_sb
                    )

                # ---- phase 3: output projection ----
                with tc.tile_pool(name="wpp", bufs=1) as wpp, \
                     tc.tile_pool(name="op", bufs=2) as op:
                    wp_sb = []
                    for c in range(NKC):
                        w = wpp.tile([128, C], F32R, tag=f"wp{c}")
                        nc.sync.dma_start(out=w, in_=w_proj[128 * c:128 * (c + 1), :])
                        wp_sb.append(w)
                    for t in range(NT):
                        ps = psum.tile([128, C], F32, tag="y")
                        for c in range(NKC):
                            for a, b in ((0, 512), (512, C)):
                                nc.tensor.matmul(
                                    ps[:, a:b],
                                    ypair[c][:, 128 * t:128 * (t + 1)],
                                    wp_sb[c][:, a:b],
                                    start=(c == 0), stop=(c == NKC - 1),
                                )
                        o_sb = op.tile([128, C], F32, tag="o")
                        nc.vector.tensor_add(o_sb, ps[:, :], bproj_bc)
                        nc.sync.dma_start(
                            out=out[128 * t:128 * (t + 1), :], in_=o_sb
                        )
    nc.finalize()
    return nc


def _get_nc():
    if "nc" not in _CACHE:
        _CACHE["nc"] = _build()
    return _CACHE["nc"]


def _prep_in_maps(x, w_attn, b_attn, w_proj, b_proj):
    x = np.asarray(x, dtype=np.float32)
    w_attn = np.asarray(w_attn, dtype=np.float32)
    b_attn = np.asarray(b_attn, dtype=np.float32)
    w_proj = np.asarray(w_proj, dtype=np.float32)
    b_proj = np.asarray(b_proj, dtype=np.float32)

    bqk = np.ascontiguousarray(b_attn[: 2 * C].reshape(2 * C // 128, 128).T)
    bv = np.ascontiguousarray(b_attn[2 * C:])
    tri = np.triu(np.ones((128, 128), dtype=np.float32))
    w_attn_c = np.ascontiguousarray(w_attn)
    w_proj_c = np.ascontiguousarray(w_proj)
    in_maps = []
    for b in range(B):
        in_maps.append({
            "xT": np.ascontiguousarray(x[b].T),
            "w_attn": w_attn_c,
            "bqk": bqk,
            "bv": bv,
            "w_proj": w_proj_c,
            "bproj": b_proj,
            "tri": tri,
        })
    return in_maps


def kernel(x, w_attn, b_attn, w_proj, b_proj):
    from concourse.bass_utils import run_bass_kernel_spmd

    nc = _get_nc()
    in_maps = _prep_in_maps(x, w_attn, b_attn, w_proj, b_proj)
    res = run_bass_kernel_spmd(nc, in_maps, core_ids=list(range(B)))
    return np.stack([res.results[b]["out"] for b in range(B)], axis=0)
